# revision 73
# baseline (speedup 1.0000x reference)
"""Distributed causal multi-head attention for TRN2 (8 NeuronCores).

Problem: B=2, T=2048, D=1024, H=16 heads (head_dim 64), causal MHA:
  q,k,v = x@W{q,k,v}+b, q *= dh**-0.5, o = softmax(mask(q k^T)) v, out = o@Wp + bp

Sharding: 8-way tensor parallel over heads.  Core r handles BOTH batches,
heads {2r, 2r+1}, and output columns 128r..128(r+1).  This makes the
AllGather a single uniform 8-core collective (the 4-core-group path
measured ~50us/call vs ~7us for 8-core) with every gathered byte used by
every core.  Per core:
  - QKV projections in fp16 on TensorE (q/k produced transposed [hd, t],
    v produced natural [t, hd] with an appended ones-column)
  - scores computed transposed [keys, q] (K=64 contraction, two heads
    packed into the 128x128 PE array via row tiling, both writing halves
    of one 2-bank PSUM tile); ONE fused exp per key tile on ScalarE
    (the pipeline pacer); causal handled by key-tile skipping + a
    post-exp 0/1 mask multiply on the diagonal blocks
  - AV uses exp-weights as the stationary operand -> o natural [q, hd]
    with per-partition row sums for free (ones column of v); normalize
    with a per-partition reciprocal; AV interleaves with scores at lag 1
    so the PE never waits long on ScalarE
  - o is PE-transposed locally (cheap) so the AllGather carries oT and
    the output projection needs no DMA transposes
  - output projection computes a 128-column slice per core, pipelined
    one chunk behind the AllGather.
Host side only shards/converts inputs, concatenates outputs, and adds
the bias terms that are mathematically output-constant (bv@Wp + bp; bk
cancels in softmax; bq is applied on device).
"""

import os
import numpy as np

B, T, D, H = 2, 2048, 1024, 16
DH = 64
NCORES = 8
HPC = H // NCORES      # heads per core = 2
CD = HPC * DH          # per-core head-dim / out columns = 128
P = 128
NCH = 4                # T chunks for the AllGather pipeline
CHUNK = T // NCH       # 512
KT = T // P            # 16 key tiles
KD = D // P            # 8 contraction tiles for the projections

_CACHE = {}

# Results of the last device run (for test harnesses): BassKernelResults
LAST_RESULT = None


def _build_nc():
    import concourse.bass as bass
    import concourse.mybir as mybir
    import concourse.tile as tile
    from concourse import bacc
    from contextlib import ExitStack

    fp = mybir.dt.float16
    f32 = mybir.dt.float32
    AF = mybir.ActivationFunctionType

    nc = bacc.Bacc("TRN2", target_bir_lowering=False, debug=False,
                   num_devices=NCORES)

    xT = nc.dram_tensor("xT", [D, B, T], fp, kind="ExternalInput").ap()
    wq = nc.dram_tensor("wq", [D, CD], fp, kind="ExternalInput").ap()
    wk = nc.dram_tensor("wk", [D, CD], fp, kind="ExternalInput").ap()
    wv = nc.dram_tensor("wv", [D, CD], fp, kind="ExternalInput").ap()
    wp = nc.dram_tensor("wp", [D, CD], fp, kind="ExternalInput").ap()
    bqp = nc.dram_tensor("bqp", [P, 1], f32, kind="ExternalInput").ap()
    maskf = nc.dram_tensor("maskf", [P, P], fp, kind="ExternalInput").ap()
    ident = nc.dram_tensor("ident", [P, P], fp, kind="ExternalInput").ap()
    out = nc.dram_tensor("out", [B, T, CD], f32, kind="ExternalOutput").ap()

    obounce = nc.dram_tensor("obounce", [NCH, B, CD, CHUNK], fp).ap()
    # chunks 0+1 share one collective (its input is ready just as the
    # warmup/skew clears); chunk 2 alone; chunk 3 per batch for the tail
    gath01 = nc.dram_tensor("gath01", [NCORES, 2, B, CD, CHUNK], fp,
                            addr_space="Shared").ap()
    gath2 = nc.dram_tensor("gath2", [NCORES, B, CD, CHUNK], fp,
                           addr_space="Shared").ap()
    gath3 = nc.dram_tensor("gath3", [NCORES, B, CD, CHUNK], fp,
                           addr_space="Shared").ap()
    warm_in = nc.dram_tensor("warm_in", [P], fp).ap()
    warm_out = nc.dram_tensor("warm_out", [NCORES, P], fp,
                              addr_space="Shared").ap()

    RG = [[0, 1, 2, 3, 4, 5, 6, 7]]

    with tile.TileContext(nc, num_cores=NCORES) as tc, ExitStack() as ctx:
        const = ctx.enter_context(tc.tile_pool(name="const", bufs=1))
        work = ctx.enter_context(tc.tile_pool(name="work", bufs=3))
        expp = ctx.enter_context(tc.tile_pool(name="expp", bufs=18))
        otkp = ctx.enter_context(tc.tile_pool(name="otkp", bufs=36))
        osbp = ctx.enter_context(tc.tile_pool(name="osbp", bufs=8))
        psum = ctx.enter_context(tc.tile_pool(name="psum", bufs=2,
                                              space="PSUM"))

        # ---- persistent SBUF ----
        xT_sb = const.tile([P, KD, B, T], fp)        # 64 KB/p
        wq_sb = const.tile([P, KD, CD], fp)
        wk_sb = const.tile([P, KD, CD], fp)
        wv_sb = const.tile([P, KD, CD], fp)
        wp_sb = const.tile([P, KD, CD], fp)
        bq_sb = const.tile([P, 1], f32)
        mask_sb = const.tile([P, P], fp)             # 0/1 lower triangle
        ident_sb = const.tile([P, P], fp)
        qT_sb = const.tile([P, B, T], fp)            # 2 heads stacked
        kT_sb = const.tile([P, B, T], fp)
        v_sb = const.tile([P, KT, B, HPC, DH + 1], fp)

        # weights first so the first projection matmuls start immediately;
        # x streams in per chunk behind them
        nc.sync.dma_start(wq_sb[:], wq.rearrange("(k p) c -> p k c", p=P))
        nc.sync.dma_start(wk_sb[:], wk.rearrange("(k p) c -> p k c", p=P))
        nc.sync.dma_start(wv_sb[:], wv.rearrange("(k p) c -> p k c", p=P))
        nc.sync.dma_start(wp_sb[:], wp.rearrange("(k p) c -> p k c", p=P))
        nc.sync.dma_start(bq_sb[:], bqp)
        nc.sync.dma_start(mask_sb[:], maskf)
        nc.sync.dma_start(ident_sb[:], ident)
        nc.vector.memset(v_sb[:, :, :, :, DH:DH + 1], 1.0)
        xT_r = xT.rearrange("(k p) b t -> p k b t", p=P)
        for t4 in range(NCH):
            # split the 8MB load so chunk-0 compute starts immediately
            for b in range(B):
                nc.sync.dma_start(
                    xT_sb[:, :, b, t4 * 512:(t4 + 1) * 512],
                    xT_r[:, :, b, t4 * 512:(t4 + 1) * 512])
        # tiny warmup collective: absorbs the first-collective latency
        # anomaly while the input DMAs stream
        nc.gpsimd.collective_compute(
            "AllGather", bass.mybir.AluOpType.bypass,
            replica_groups=RG, ins=[warm_in], outs=[warm_out])

        def qkv_units(t4):
            """Projection work for T-chunk t4 as a list of closures, so
            it can be drip-fed into the attention k-loop (fills the PE
            while ScalarE paces the exp pipeline)."""
            units = []

            psqk_box = {}

            def q_unit(b):
                psqk = psum.tile([P, 1024], f32, tag="big", bufs=3,
                                 name=f"psqk_{t4}_{b}")
                psqk_box[b] = psqk
                for k in range(KD):
                    nc.tensor.matmul(
                        psqk[:, 0:512], wq_sb[:, k, :],
                        xT_sb[:, k, b, t4 * 512:(t4 + 1) * 512],
                        start=(k == 0), stop=(k == KD - 1))
                nc.vector.tensor_scalar_add(
                    qT_sb[:, b, t4 * 512:(t4 + 1) * 512], psqk[:, 0:512],
                    bq_sb[:, 0:1])

            def k_unit(b):
                psqk = psqk_box[b]
                for k in range(KD):
                    nc.tensor.matmul(
                        psqk[:, 512:1024], wk_sb[:, k, :],
                        xT_sb[:, k, b, t4 * 512:(t4 + 1) * 512],
                        start=(k == 0), stop=(k == KD - 1))
                nc.vector.tensor_copy(
                    kT_sb[:, b, t4 * 512:(t4 + 1) * 512], psqk[:, 512:1024])

            def v_unit(b, tt):
                psv = psum.tile([P, 256], f32, tag="big", bufs=3,
                                name=f"psv_{tt}_{b}")
                for k in range(KD):
                    nc.tensor.matmul(
                        psv[:, :CD], xT_sb[:, k, b, tt * P:(tt + 1) * P],
                        wv_sb[:, k, :], start=(k == 0),
                        stop=(k == KD - 1))
                nc.vector.tensor_copy(
                    out=v_sb[:, tt, b, :, 0:DH],
                    in_=psv[:, :CD].rearrange("p (h d) -> p h d", h=HPC))

            for b in range(B):
                units.append((t4, b, lambda b=b: q_unit(b)))
                units.append((t4, b, lambda b=b: k_unit(b)))
                for tt in range(4 * t4, 4 * t4 + 4):
                    units.append((t4, b, lambda b=b, tt=tt: v_unit(b, tt)))
            return units

        def qkv_chunk(t4):
            for _, _, u in qkv_units(t4):
                u()

        def attention_batch(c, b, filler=None):
            """Causal attention for q-chunk c, batch b (2 heads packed).

            scores for both heads go into one [128,1024] PSUM tile
            (row-packed K=64 matmuls -> halves), one fused exp per key
            tile, AV interleaved with lag 1.  AV accumulates all four
            q-subtiles of each head in one PSUM bank (4 interleaved
            accumulation groups as column ranges)."""
            nkt = 4 * (c + 1)
            exp_tiles = {}
            pso = {}
            for hh in range(2):
                pso[hh] = psum.tile([P, 4, DH + 1], f32, tag="o",
                                    name=f"pso_{c}_{b}_{hh}")

            def do_scores(k):
                ps_s = psum.tile([P, 1024], f32, tag="big", bufs=3,
                                 name=f"ps_{c}_{b}_{k}")
                for hh in range(2):
                    lo, hi = hh * DH, (hh + 1) * DH
                    nc.tensor.matmul(
                        ps_s[:, hh * 512:(hh + 1) * 512],
                        kT_sb[lo:hi, b, k * P:(k + 1) * P],
                        qT_sb[lo:hi, b, c * 512:(c + 1) * 512],
                        start=True, stop=True)
                e = expp.tile([P, 1024], fp, tag="expT",
                              name=f"expT_{c}_{b}_{k}")
                j = k - 4 * c
                if j >= 2:
                    # deep diagonal tile: skip exp on the (never-read)
                    # below-diagonal columns - ScalarE is the pacer
                    for hh in range(2):
                        lo = hh * 512 + j * P
                        hi = (hh + 1) * 512
                        nc.scalar.activation(e[:, lo:hi], ps_s[:, lo:hi],
                                             AF.Exp)
                else:
                    nc.scalar.activation(e[:], ps_s[:], AF.Exp)
                if j >= 0:
                    blks = e[:].rearrange("p (hh q) -> p hh q", hh=2)[
                        :, :, j * P:(j + 1) * P]
                    nc.vector.tensor_mul(
                        blks, blks,
                        mask_sb[:, None, :].to_broadcast([P, 2, P]))
                exp_tiles[k] = e

            def do_av(k):
                # pso[hh] holds 4 interleaved accumulation groups in one
                # PSUM bank; only the first write of the bank (k==0,s==0)
                # may set start (bank-wide has_written clear).
                for hh in range(2):
                    h = 2 * b + hh  # local index only
                    for s in range(4):
                        if k <= 4 * c + s:
                            nc.tensor.matmul(
                                pso[hh][:, s, :],
                                exp_tiles[k][:, hh * 512 + s * P:
                                             hh * 512 + (s + 1) * P],
                                v_sb[:, k, b, hh, :],
                                start=(k == 0 and s == 0),
                                stop=(k == 4 * c + s),
                                skip_group_check=True)

            for k in range(nkt + 1):
                if k < nkt:
                    do_scores(k)
                if k > 0:
                    do_av(k - 1)
                if filler is not None and 2 <= k < nkt - 1:
                    # no fills near the end of the loop: the AG-critical
                    # finish work must not queue behind drip units
                    filler()
            return pso

        def proj_loads(c):
            """Plain DMA loads of the gathered (already transposed) heads."""
            oTk = {}
            for b2 in range(B):
                for k in range(KD):
                    t_ = otkp.tile([P, CHUNK], fp, tag="oTk",
                                   name=f"oTk_{c}_{b2}_{k}")
                    src = gath01[k, c, b2] if c < 2 else gath2[k, b2]
                    nc.sync.dma_start(t_[:], src)
                    oTk[(b2, k)] = t_
            return oTk

        def proj_batch(c, b2, oTk):
            """Output projection for T-chunk c, one batch (one PSUM bank
            holding 4 interleaved q-subtile accumulation groups)."""
            psp = psum.tile([P, 4, CD], f32, tag="big", bufs=3,
                            name=f"psp_{c}_{b2}")
            for k in range(KD):
                for s in range(4):
                    nc.tensor.matmul(
                        psp[:, s, :], oTk[(b2, k)][:, s * P:(s + 1) * P],
                        wp_sb[:, k, :],
                        start=(k == 0 and s == 0), stop=(k == KD - 1),
                        skip_group_check=True)
            outsb = work.tile([P, 4, CD], f32, tag="outsb",
                              name=f"outsb_{c}_{b2}")
            nc.vector.tensor_copy(outsb[:], psp[:])
            nc.sync.dma_start(
                out[b2, c * 512:(c + 1) * 512, :].rearrange(
                    "(s p) col -> p s col", p=P),
                outsb[:])

        def proj_chunk(c, oTk):
            for b2 in range(B):
                proj_batch(c, b2, oTk)

        def finish_batch(c, b, pso):
            """normalize -> PE transpose -> bounce buffer for AG."""
            osb = osbp.tile([P, 4, CD], fp, tag="osb",
                            name=f"osb_{c}_{b}")
            for hh in range(2):
                for s in range(4):
                    rec = work.tile([P, 1], f32, tag="rec",
                                    name=f"rec_{c}_{b}_{hh}_{s}")
                    nc.vector.reciprocal(rec[:],
                                         pso[hh][:, s, DH:DH + 1])
                    nc.vector.tensor_scalar_mul(
                        osb[:, s, hh * DH:(hh + 1) * DH],
                        pso[hh][:, s, 0:DH], rec[:])
            # local PE transpose: obounce carries oT so the projection
            # needs no DMA transposes
            obT = work.tile([P, 4, P], fp, tag="obT", name=f"obT_{c}_{b}")
            for s in range(4):
                trp = psum.tile([P, P], fp, tag="o",
                                name=f"trp_{c}_{b}_{s}")
                nc.tensor.transpose(trp[:], osb[:, s, :], ident_sb[:])
                nc.vector.tensor_copy(obT[:, s, :], trp[:])
            nc.sync.dma_start(
                obounce[c, b].rearrange("p (s t) -> p s t", t=P), obT[:])

        # pipeline: attention(c) paces ScalarE; AllGathers fly while later
        # chunks compute (chunks 0+1 share one collective, chunk 3 goes
        # per batch to shrink the tail); qkv(c+1) and ready projection
        # work are drip-fed INTO the attention k-loop so the PE fills
        # ScalarE-paced slack instead of idling afterwards.
        oTks = {}
        pending = []   # (chunk, batch, closure) projection units

        def filler():
            if pending:
                pending.pop(0)[2]()
            if len(pending) > 8:
                pending.pop(0)[2]()

        def drain_for(c, b):
            # emit every unit attention(c, b) depends on (its own chunk's
            # batch-b projections); later units keep dripping
            while any(t == c and bb == b for t, bb, _ in pending):
                pending.pop(0)[2]()

        # only batch 0's projections block the first scores
        units0 = qkv_units(0)
        for _, _, u in units0[:6]:
            u()
        pending.extend(units0[6:])
        for c in range(NCH):
            if c == 3:
                # AG(0+1) is long done; prefetch its gathered tiles
                oTks[0] = proj_loads(0)
                oTks[1] = proj_loads(1)
            pending.extend(qkv_units(c + 1) if c + 1 < NCH else [])
            for b in range(B):
                drain_for(c, b)
                pso = attention_batch(c, b, filler=filler)
                finish_batch(c, b, pso)
            if c == 1:
                nc.gpsimd.collective_compute(
                    "AllGather", bass.mybir.AluOpType.bypass,
                    replica_groups=RG,
                    ins=[obounce[0:2]],
                    outs=[gath01])
            elif c == 2:
                nc.gpsimd.collective_compute(
                    "AllGather", bass.mybir.AluOpType.bypass,
                    replica_groups=RG,
                    ins=[obounce[2]],
                    outs=[gath2])
            elif c == 3:
                nc.gpsimd.collective_compute(
                    "AllGather", bass.mybir.AluOpType.bypass,
                    replica_groups=RG,
                    ins=[obounce[3]],
                    outs=[gath3])
        while pending:
            pending.pop(0)[2]()
        # projection tail: chunks 0-2 fill the PE while chunk 3's
        # gather completes
        proj_chunk(0, oTks[0])
        proj_chunk(1, oTks[1])
        proj_chunk(2, proj_loads(2))
        oTk3 = {}
        for b2 in range(B):
            for k in range(KD):
                t_ = otkp.tile([P, CHUNK], fp, tag="oTk",
                               name=f"oTk3_{b2}_{k}")
                nc.sync.dma_start(t_[:], gath3[k, b2])
                oTk3[(b2, k)] = t_
        proj_chunk(NCH - 1, oTk3)

    nc.finalize()
    return nc


def _get_nc():
    if "nc" not in _CACHE:
        _CACHE["nc"] = _build_nc()
    return _CACHE["nc"]


def kernel(x, Wq, bq, Wk, bk, Wv, bv, Wp, bp):
    global LAST_RESULT
    from concourse.bass_utils import run_bass_kernel_spmd

    x = np.asarray(x, dtype=np.float32)
    Wq = np.asarray(Wq, dtype=np.float32)
    Wk = np.asarray(Wk, dtype=np.float32)
    Wv = np.asarray(Wv, dtype=np.float32)
    Wp = np.asarray(Wp, dtype=np.float32)
    bq = np.asarray(bq, dtype=np.float32)
    bv = np.asarray(bv, dtype=np.float32)
    bp = np.asarray(bp, dtype=np.float32)

    s = DH ** -0.5
    maskf = np.where(
        np.arange(P)[:, None] <= np.arange(P)[None, :], 1.0, 0.0
    ).astype(np.float16)
    ident = np.eye(P, dtype=np.float16)
    xTg = np.ascontiguousarray(np.stack([x[0].T, x[1].T], axis=1)
                               ).astype(np.float16)

    in_maps = []
    for r in range(NCORES):
        cols = slice(r * CD, (r + 1) * CD)
        in_maps.append({
            "xT": xTg,
            "wq": (Wq[:, cols] * s).astype(np.float16),
            "wk": np.ascontiguousarray(Wk[:, cols]).astype(np.float16),
            "wv": np.ascontiguousarray(Wv[:, cols]).astype(np.float16),
            "wp": np.ascontiguousarray(Wp[:, cols]).astype(np.float16),
            "bqp": np.ascontiguousarray((bq[cols] * s).reshape(P, 1)),
            "maskf": maskf,
            "ident": ident,
        })

    nc = _get_nc()
    res = run_bass_kernel_spmd(
        nc, in_maps, core_ids=list(range(NCORES)),
        trace=bool(int(os.environ.get("KERNEL_TRACE", "0"))))
    LAST_RESULT = res

    out = np.empty((B, T, D), dtype=np.float32)
    for r in range(NCORES):
        out[:, :, r * CD:(r + 1) * CD] = res.results[r]["out"]
    # bias terms that are constant w.r.t. the data path:
    #   v-bias passes through softmax rows (sum=1) -> + bv@Wp; plus bp.
    #   (bk shifts every logit in a row equally -> cancels in softmax.)
    out += (bv @ Wp + bp)[None, None, :]
    return out
